# revision 41
# baseline (speedup 1.0000x reference)
"""Trainium2 Bass kernel for the Dedicom decoder problem.

Math: score_b = (z[e0]*d) @ W @ (z[e1]*d) = z[e0] @ (diag(d) W diag(d)) @ z[e1].
All-pairs scores S = (Z @ diag(d) W diag(d)) @ Z^T contain every edge score.
Each core computes 512 rows of S (its e0 block); its copy of Z^T (fp8) is
column-rotated so the core's own block sits at columns 0:512, which lets the
A-phase reuse the same zt8 tensor as its moving operand (no separate zb):
  W'8 = fp8(diag(d) @ W)                     on-chip scale+cast
  A   = Z_blk @ W'                           fp8 DoubleRow matmuls
  S   = (A * d * 128) @ Z^T                  fp8 DoubleRow matmuls (2^7 scale)
S lands in SBUF as a bf16 table [128 part, 4 mt, 4096] via ACT/DVE psum
copies.  Per-edge extraction runs on the gpsimd engine: one indirect_copy per
(row-tile, column-half) gathers each edge's f32-aligned bf16 PAIR from the
f32-bitcast view of the table (per-16-partition-group slot lists built on
host).  ACT applies sigmoid(x/128) to the bf16 view of the gathered granules;
the host picks each edge's half of its pair by pure indexing.
"""

import numpy as np
import ml_dtypes

BF = ml_dtypes.bfloat16
F8 = ml_dtypes.float8_e4m3

N_DRUGS = 4096
D = 512
N_CORES = 8
BLK = N_DRUGS // N_CORES   # 512 rows of S per core
MT = BLK // 128            # 4 row tiles
SSCALE = 128.0             # 2^7 fp8 dynamic-range scale on A*d

_cache = {}


def _build(I):
    """Build + compile the SPMD program; I = slots per (mt, col-half) x2."""
    import concourse.bass as bass  # noqa: F401
    import concourse.bacc as bacc
    import concourse.mybir as mybir
    import concourse.tile as tile

    f32 = mybir.dt.float32
    bf16 = mybir.dt.bfloat16
    fp8 = mybir.dt.float8e4
    u16 = mybir.dt.uint16
    DR = mybir.MatmulPerfMode.DoubleRow
    SIG = mybir.ActivationFunctionType.Sigmoid
    IH = I // 2          # slots per (mt, column-half, group)
    IWH = IH // 16

    nc = bacc.Bacc("TRN2", target_bir_lowering=False, debug=False,
                   num_devices=N_CORES)

    ZT8 = nc.dram_tensor("zt8", [128, 2, 2, N_DRUGS], fp8, kind="ExternalInput")
    WT = nc.dram_tensor("w", [128, 4, D], fp8, kind="ExternalInput")
    DV = nc.dram_tensor("dv", [128, 8], f32, kind="ExternalInput")
    IX = nc.dram_tensor("idx", [128, MT * 2 * IWH], u16, kind="ExternalInput")
    OUT = nc.dram_tensor("out", [MT, 2, 128, 2 * IH], bf16, kind="ExternalOutput")

    with tile.TileContext(nc) as tc:
        with (
            tc.tile_pool(name="sb", bufs=1) as sb,
            tc.tile_pool(name="psum", bufs=4, space="PSUM") as psum,
        ):
            # PE p-state warmup + act-func-set (sigmoid+copy) prefetch
            wu_sb = sb.tile([128, 512], bf16)
            nc.gpsimd.memset(wu_sb[:], 0.0)
            pw = psum.tile([128, 1024], f32, tag="ps", name="warm")
            for i in range(8):
                nc.tensor.matmul(pw[:, :512], wu_sb[:, :128], wu_sb[:],
                                 start=True, stop=True)

            # input DMAs spread across the SP/ACT/DVE HWDGE rings: each
            # dma_start costs ~650ns of serial sequencer time per ring
            w_sb = sb.tile([128, 4, D], fp8)
            nc.sync.dma_start(w_sb[:], WT.ap())
            dv_sb = sb.tile([128, 8], f32)
            nc.scalar.dma_start(dv_sb[:], DV.ap())
            zt_sb = sb.tile([128, 2, 2, N_DRUGS], fp8)
            nc.sync.dma_start(zt_sb[:, :, :, :512], ZT8.ap()[:, :, :, :512])
            nc.sync.dma_start(zt_sb[:, :, :, 512:2048],
                              ZT8.ap()[:, :, :, 512:2048])
            nc.sync.dma_start(zt_sb[:, :, :, 2048:], ZT8.ap()[:, :, :, 2048:])
            ix_sb = sb.tile([128, MT * 2 * IWH], u16)
            nc.scalar.dma_start(ix_sb[:], IX.ap())

            # W'8 = fp8(128 * diag(d) @ W): the 2^7 scale keeps W*d out of
            # fp8's denormal range; it cancels in the a8 scale (just d).
            # dv columns 4:8 hold host-prescaled d*8 (lossless 2^3 shift).
            w8 = sb.tile([128, 2, 2, D], fp8)
            for jc in range(4):
                nc.vector.tensor_scalar_mul(w8[:, jc // 2, jc % 2, :],
                                            w_sb[:, jc, :],
                                            dv_sb[:, 4 + jc:5 + jc])

            # A-phase (fp8 DoubleRow): pa[kc][x, m] = 128*A[m, kc*128+x]
            # a8[p, kc2, i, m] = A[m, k]*d[k]*128 (fp8e4), k = kc2*256+i*128+p
            a8 = sb.tile([128, 2, 2, BLK], fp8)
            pa0 = psum.tile([128, 1024], f32, tag="ps", name="pa0")
            pa1 = psum.tile([128, 1024], f32, tag="ps", name="pa1")
            pas = (pa0, pa0, pa1, pa1)
            for kc in range(4):
                for jch in range(2):
                    nc.tensor.matmul(
                        pas[kc][:, (kc % 2) * 512:(kc % 2 + 1) * 512],
                        w8[:, jch, :, kc * 128:(kc + 1) * 128],
                        zt_sb[:, jch, :, :512],
                        start=(jch == 0), stop=(jch == 1), perf_mode=DR)
            for kc in range(4):
                src_ap = pas[kc][:, (kc % 2) * 512:(kc % 2 + 1) * 512]
                dst_ap = a8[:, kc // 2, kc % 2, :]
                if kc >= 2:
                    nc.scalar.mul(dst_ap, src_ap, dv_sb[:, kc:kc + 1])
                else:
                    nc.vector.tensor_scalar_mul(dst_ap, src_ap,
                                                dv_sb[:, kc:kc + 1])

            # force the sigmoid act-func-set load into ACT's idle window
            # here (it also contains 'copy', so no reload later)
            wu2 = sb.tile([128, 32], bf16)
            nc.scalar.activation(wu2[:], wu_sb[:, :32], SIG)

            # S-phase (fp8 DoubleRow): s_sb[p, mt, n] = S[mt*128+p, n] bf16
            s_sb = sb.tile([128, MT, N_DRUGS], bf16)
            x_sb = sb.tile([128, MT, I], f32)
            y_sb = sb.tile([128, MT, 2 * I], bf16)
            # ch-major: all column-half-0 quarters first (they only need
            # the first two zt8 column DMAs), then column-half 1; gathers
            # follow each (mt, ch) pair so Pool is fed without stalls
            GORDER = [(0, 0), (1, 0), (2, 0), (3, 0),
                      (0, 1), (1, 1), (2, 1), (3, 1)]
            for mt, ch in GORDER:
                pss = [psum.tile([128, 1024], f32, tag="ps",
                                 name=f"s_{mt}_{ch}_{qh}") for qh in range(2)]
                for qh in range(2):
                    for kc2 in range(2):
                        qt = 2 * ch + qh
                        for nch in range(2 * qt, 2 * qt + 2):
                            nc.tensor.matmul(
                                pss[qh][:, (nch % 2) * 512:(nch % 2 + 1) * 512],
                                a8[:, kc2, :, mt * 128:(mt + 1) * 128],
                                zt_sb[:, kc2, :, nch * 512:(nch + 1) * 512],
                                start=(kc2 == 0), stop=(kc2 == 1),
                                perf_mode=DR)
                for qh in range(2):
                    qt = 2 * ch + qh
                    dst = s_sb[:, mt, qt * 1024:(qt + 1) * 1024]
                    if qh == 1:
                        nc.scalar.copy(dst, pss[qh][:])
                    else:
                        nc.vector.tensor_copy(dst, pss[qh][:])
                nc.gpsimd.indirect_copy(
                    x_sb[:, mt, ch * IH:(ch + 1) * IH],
                    s_sb[:, mt, ch * 2048:(ch + 1) * 2048].bitcast(f32),
                    ix_sb[:, (mt * 2 + ch) * IWH:(mt * 2 + ch + 1) * IWH],
                    True)

            for mt, ch in GORDER:
                ysl = y_sb[:, mt, 2 * ch * IH:2 * (ch + 1) * IH]
                nc.scalar.activation(
                    ysl, x_sb[:, mt, ch * IH:(ch + 1) * IH].bitcast(bf16),
                    SIG, scale=1.0 / SSCALE)
                if (mt, ch) == (3, 1):
                    nc.scalar.dma_start(OUT.ap()[mt, ch], ysl)
                else:
                    nc.sync.dma_start(OUT.ap()[mt, ch], ysl)

    nc.compile()
    return nc


def _get_program(I):
    if I not in _cache:
        _cache[I] = _build(I)
    return _cache[I]


def kernel(z_drug, global_weight, local_diag, batch_edges, edge_sub_type_idx,
           **_unused):
    from concourse.bass_utils import run_bass_kernel_spmd

    z = np.asarray(z_drug, np.float32)
    W = np.asarray(global_weight, np.float32)
    ld = np.asarray(local_diag, np.float32)
    e = np.asarray(batch_edges)
    sub = int(np.asarray(edge_sub_type_idx))
    d = ld[sub]
    B = e.shape[1]
    e0 = e[0].astype(np.int64)
    e1 = e[1].astype(np.int64)

    # shared layouts; zt8 is rotated per core below
    zt8 = np.ascontiguousarray(
        z.T.reshape(2, 2, 128, N_DRUGS).transpose(2, 0, 1, 3)).astype(F8)
    wt = np.ascontiguousarray(
        W.reshape(4, 128, D).transpose(1, 0, 2) * 16.0).astype(F8)
    dvb = d.reshape(4, 128).T.astype(np.float32)
    dv = np.ascontiguousarray(np.concatenate([dvb, dvb * np.float32(8.0)],
                                             axis=1))

    core = e0 // BLK
    r = e0 - core * BLK
    p = r % 128
    mt = r // 128
    g = p // 16
    e1r = (e1 - core * BLK) % N_DRUGS    # column index in rotated zt8
    ch = (e1r >= 2048).astype(np.int64)

    # slot capacity IH: max edges per (core, mt, colhalf, group), mult of 32
    cell = (((core * MT + mt) * 2 + ch) * 8 + g).astype(np.int64)
    counts = np.bincount(cell, minlength=N_CORES * MT * 2 * 8)
    IH = max(32, int(-(-counts.max() // 32)) * 32)
    I = 2 * IH
    IWH = IH // 16

    # slot index within each cell, in edge order
    order = np.argsort(cell, kind="stable")
    slot = np.empty(B, np.int64)
    arange = np.arange(B, dtype=np.int64)
    cs = np.concatenate([[0], np.cumsum(counts)])
    slot[order] = arange - cs[cell[order]]

    gran = ((e1r % 2048) >> 1).astype(np.uint16)  # f32 idx within col half
    half = (e1r & 1).astype(np.int64)    # which bf16 half of the granule

    in_maps = []
    for c in range(N_CORES):
        m = core == c
        idx = np.zeros((128, MT * 2 * IWH), np.uint16)
        # idx[16g+q, (mt*2+ch)*IWH + s] = granule of the cell slot (s*16+q)
        q = slot[m] % 16
        s = slot[m] // 16
        idx[16 * g[m] + q, (mt[m] * 2 + ch[m]) * IWH + s] = gran[m]
        ztc = np.roll(zt8, -c * BLK, axis=3) if c else zt8
        in_maps.append({"zt8": np.ascontiguousarray(ztc), "w": wt, "dv": dv,
                        "idx": idx})

    nc = _get_program(I)
    res = run_bass_kernel_spmd(nc, in_maps, list(range(N_CORES)))

    out = np.empty(B, np.float32)
    for c in range(N_CORES):
        m = core == c
        Y = np.asarray(res.results[c]["out"])  # [MT, 2, 128, 2*IH] bf16
        out[m] = Y[mt[m], ch[m], p[m],
                   2 * slot[m] + half[m]].astype(np.float32)
    return out


if __name__ == "__main__":
    dat = np.load("/root/problem/cached_io.npz")
    inputs = {k: dat[k] for k in ("z_drug", "global_weight", "local_diag",
                                  "batch_edges", "edge_sub_type_idx")}
    expected = dat["expected"]
    actual = kernel(**inputs)
    err = np.abs(actual - expected)
    print("max abs err:", err.max(), "mean:", err.mean())
    print("Relative error:", err.max() / np.abs(expected).max())


# revision 42
# speedup vs baseline: 1.0056x; 1.0056x over previous
"""Trainium2 Bass kernel for the Dedicom decoder problem.

Math: score_b = (z[e0]*d) @ W @ (z[e1]*d) = z[e0] @ (diag(d) W diag(d)) @ z[e1].
All-pairs scores S = (Z @ diag(d) W diag(d)) @ Z^T contain every edge score.
Each core computes 512 rows of S (its e0 block); its copy of Z^T (fp8) is
column-rotated so the core's own block sits at columns 0:512, which lets the
A-phase reuse the same zt8 tensor as its moving operand (no separate zb):
  W'8 = fp8(diag(d) @ W)                     on-chip scale+cast
  A   = Z_blk @ W'                           fp8 DoubleRow matmuls
  S   = (A * d * 128) @ Z^T                  fp8 DoubleRow matmuls (2^7 scale)
S lands in SBUF as a bf16 table [128 part, 4 mt, 4096] via ACT/DVE psum
copies.  Per-edge extraction runs on the gpsimd engine: one indirect_copy per
(row-tile, column-half) gathers each edge's f32-aligned bf16 PAIR from the
f32-bitcast view of the table (per-16-partition-group slot lists built on
host).  ACT applies sigmoid(x/128) to the bf16 view of the gathered granules;
the host picks each edge's half of its pair by pure indexing.
"""

import numpy as np
import ml_dtypes

BF = ml_dtypes.bfloat16
F8 = ml_dtypes.float8_e4m3

N_DRUGS = 4096
D = 512
N_CORES = 8
BLK = N_DRUGS // N_CORES   # 512 rows of S per core
MT = BLK // 128            # 4 row tiles
SSCALE = 128.0             # 2^7 fp8 dynamic-range scale on A*d

_cache = {}


def _build(I):
    """Build + compile the SPMD program; I = slots per (mt, col-half) x2."""
    import concourse.bass as bass  # noqa: F401
    import concourse.bacc as bacc
    import concourse.mybir as mybir
    import concourse.tile as tile

    f32 = mybir.dt.float32
    bf16 = mybir.dt.bfloat16
    fp8 = mybir.dt.float8e4
    u16 = mybir.dt.uint16
    DR = mybir.MatmulPerfMode.DoubleRow
    SIG = mybir.ActivationFunctionType.Sigmoid
    IH = I // 2          # slots per (mt, column-half, group)
    IWH = IH // 16

    nc = bacc.Bacc("TRN2", target_bir_lowering=False, debug=False,
                   num_devices=N_CORES)

    ZT8 = nc.dram_tensor("zt8", [128, 2, 2, N_DRUGS], fp8, kind="ExternalInput")
    WT = nc.dram_tensor("w", [128, 4, D], fp8, kind="ExternalInput")
    DV = nc.dram_tensor("dv", [128, 8], f32, kind="ExternalInput")
    IX = nc.dram_tensor("idx", [128, MT * 2 * IWH], u16, kind="ExternalInput")
    OUT = nc.dram_tensor("out", [MT, 2, 128, 2 * IH], bf16, kind="ExternalOutput")

    with tile.TileContext(nc) as tc:
        with (
            tc.tile_pool(name="sb", bufs=1) as sb,
            tc.tile_pool(name="psum", bufs=4, space="PSUM") as psum,
        ):
            # PE p-state warmup + act-func-set (sigmoid+copy) prefetch
            wu_sb = sb.tile([128, 512], bf16)
            nc.gpsimd.memset(wu_sb[:], 0.0)
            pw = psum.tile([128, 1024], f32, tag="ps", name="warm")
            for i in range(8):
                nc.tensor.matmul(pw[:, :512], wu_sb[:, :128], wu_sb[:],
                                 start=True, stop=True)

            # input DMAs spread across the SP/ACT/DVE HWDGE rings: each
            # dma_start costs ~650ns of serial sequencer time per ring
            w_sb = sb.tile([128, 4, D], fp8)
            nc.sync.dma_start(w_sb[:], WT.ap())
            dv_sb = sb.tile([128, 8], f32)
            nc.scalar.dma_start(dv_sb[:], DV.ap())
            zt_sb = sb.tile([128, 2, 2, N_DRUGS], fp8)
            nc.sync.dma_start(zt_sb[:, :, :, :512], ZT8.ap()[:, :, :, :512])
            nc.sync.dma_start(zt_sb[:, :, :, 512:2048],
                              ZT8.ap()[:, :, :, 512:2048])
            nc.sync.dma_start(zt_sb[:, :, :, 2048:], ZT8.ap()[:, :, :, 2048:])
            ix_sb = sb.tile([128, MT * 2 * IWH], u16)
            nc.scalar.dma_start(ix_sb[:], IX.ap())

            # W'8 = fp8(128 * diag(d) @ W): the 2^7 scale keeps W*d out of
            # fp8's denormal range; it cancels in the a8 scale (just d).
            # dv columns 4:8 hold host-prescaled d*8 (lossless 2^3 shift).
            w8 = sb.tile([128, 2, 2, D], fp8)
            for jc in range(4):
                nc.vector.tensor_scalar_mul(w8[:, jc // 2, jc % 2, :],
                                            w_sb[:, jc, :],
                                            dv_sb[:, 4 + jc:5 + jc])

            # A-phase (fp8 DoubleRow): pa[kc][x, m] = 128*A[m, kc*128+x]
            # a8[p, kc2, i, m] = A[m, k]*d[k]*128 (fp8e4), k = kc2*256+i*128+p
            a8 = sb.tile([128, 2, 2, BLK], fp8)
            pa0 = psum.tile([128, 1024], f32, tag="ps", name="pa0")
            pa1 = psum.tile([128, 1024], f32, tag="ps", name="pa1")
            pas = (pa0, pa0, pa1, pa1)
            for kc in range(4):
                for jch in range(2):
                    nc.tensor.matmul(
                        pas[kc][:, (kc % 2) * 512:(kc % 2 + 1) * 512],
                        w8[:, jch, :, kc * 128:(kc + 1) * 128],
                        zt_sb[:, jch, :, :512],
                        start=(jch == 0), stop=(jch == 1), perf_mode=DR)
            for kc in range(4):
                src_ap = pas[kc][:, (kc % 2) * 512:(kc % 2 + 1) * 512]
                dst_ap = a8[:, kc // 2, kc % 2, :]
                if kc >= 2:
                    nc.scalar.mul(dst_ap, src_ap, dv_sb[:, kc:kc + 1])
                else:
                    nc.vector.tensor_scalar_mul(dst_ap, src_ap,
                                                dv_sb[:, kc:kc + 1])

            # force the sigmoid act-func-set load into ACT's idle window
            # here (it also contains 'copy', so no reload later)
            wu2 = sb.tile([128, 32], bf16)
            nc.scalar.activation(wu2[:], wu_sb[:, :32], SIG)

            # S-phase (fp8 DoubleRow): s_sb[p, mt, n] = S[mt*128+p, n] bf16
            s_sb = sb.tile([128, MT, N_DRUGS], bf16)
            x_sb = sb.tile([128, MT, I], f32)
            y_sb = sb.tile([128, MT, 2 * I], bf16)
            # ch-major: all column-half-0 quarters first (they only need
            # the first two zt8 column DMAs), then column-half 1; gathers
            # follow each (mt, ch) pair so Pool is fed without stalls
            GORDER = [(0, 0), (1, 0), (2, 0), (3, 0),
                      (0, 1), (1, 1), (2, 1), (3, 1)]
            for mt, ch in GORDER:
                pss = [psum.tile([128, 1024], f32, tag="ps",
                                 name=f"s_{mt}_{ch}_{qh}") for qh in range(2)]
                for qh in range(2):
                    for kc2 in range(2):
                        qt = 2 * ch + qh
                        for nch in range(2 * qt, 2 * qt + 2):
                            nc.tensor.matmul(
                                pss[qh][:, (nch % 2) * 512:(nch % 2 + 1) * 512],
                                a8[:, kc2, :, mt * 128:(mt + 1) * 128],
                                zt_sb[:, kc2, :, nch * 512:(nch + 1) * 512],
                                start=(kc2 == 0), stop=(kc2 == 1),
                                perf_mode=DR)
                for qh in range(2):
                    qt = 2 * ch + qh
                    dst = s_sb[:, mt, qt * 1024:(qt + 1) * 1024]
                    if qh == 1:
                        nc.scalar.copy(dst, pss[qh][:])
                    else:
                        nc.vector.tensor_copy(dst, pss[qh][:])
                nc.gpsimd.indirect_copy(
                    x_sb[:, mt, ch * IH:(ch + 1) * IH],
                    s_sb[:, mt, ch * 2048:(ch + 1) * 2048].bitcast(f32),
                    ix_sb[:, (mt * 2 + ch) * IWH:(mt * 2 + ch + 1) * IWH],
                    True)

            for mt, ch in GORDER:
                ysl = y_sb[:, mt, 2 * ch * IH:2 * (ch + 1) * IH]
                nc.scalar.activation(
                    ysl, x_sb[:, mt, ch * IH:(ch + 1) * IH].bitcast(bf16),
                    SIG, scale=1.0 / SSCALE)
                nc.sync.dma_start(OUT.ap()[mt, ch], ysl)

    nc.compile()
    return nc


def _get_program(I):
    if I not in _cache:
        _cache[I] = _build(I)
    return _cache[I]


def kernel(z_drug, global_weight, local_diag, batch_edges, edge_sub_type_idx,
           **_unused):
    from concourse.bass_utils import run_bass_kernel_spmd

    z = np.asarray(z_drug, np.float32)
    W = np.asarray(global_weight, np.float32)
    ld = np.asarray(local_diag, np.float32)
    e = np.asarray(batch_edges)
    sub = int(np.asarray(edge_sub_type_idx))
    d = ld[sub]
    B = e.shape[1]
    e0 = e[0].astype(np.int64)
    e1 = e[1].astype(np.int64)

    # shared layouts; zt8 is rotated per core below
    zt8 = np.ascontiguousarray(
        z.T.reshape(2, 2, 128, N_DRUGS).transpose(2, 0, 1, 3)).astype(F8)
    wt = np.ascontiguousarray(
        W.reshape(4, 128, D).transpose(1, 0, 2) * 16.0).astype(F8)
    dvb = d.reshape(4, 128).T.astype(np.float32)
    dv = np.ascontiguousarray(np.concatenate([dvb, dvb * np.float32(8.0)],
                                             axis=1))

    core = e0 // BLK
    r = e0 - core * BLK
    p = r % 128
    mt = r // 128
    g = p // 16
    e1r = (e1 - core * BLK) % N_DRUGS    # column index in rotated zt8
    ch = (e1r >= 2048).astype(np.int64)

    # slot capacity IH: max edges per (core, mt, colhalf, group), mult of 32
    cell = (((core * MT + mt) * 2 + ch) * 8 + g).astype(np.int64)
    counts = np.bincount(cell, minlength=N_CORES * MT * 2 * 8)
    IH = max(32, int(-(-counts.max() // 32)) * 32)
    I = 2 * IH
    IWH = IH // 16

    # slot index within each cell, in edge order
    order = np.argsort(cell, kind="stable")
    slot = np.empty(B, np.int64)
    arange = np.arange(B, dtype=np.int64)
    cs = np.concatenate([[0], np.cumsum(counts)])
    slot[order] = arange - cs[cell[order]]

    gran = ((e1r % 2048) >> 1).astype(np.uint16)  # f32 idx within col half
    half = (e1r & 1).astype(np.int64)    # which bf16 half of the granule

    in_maps = []
    for c in range(N_CORES):
        m = core == c
        idx = np.zeros((128, MT * 2 * IWH), np.uint16)
        # idx[16g+q, (mt*2+ch)*IWH + s] = granule of the cell slot (s*16+q)
        q = slot[m] % 16
        s = slot[m] // 16
        idx[16 * g[m] + q, (mt[m] * 2 + ch[m]) * IWH + s] = gran[m]
        ztc = np.roll(zt8, -c * BLK, axis=3) if c else zt8
        in_maps.append({"zt8": np.ascontiguousarray(ztc), "w": wt, "dv": dv,
                        "idx": idx})

    nc = _get_program(I)
    res = run_bass_kernel_spmd(nc, in_maps, list(range(N_CORES)))

    out = np.empty(B, np.float32)
    for c in range(N_CORES):
        m = core == c
        Y = np.asarray(res.results[c]["out"])  # [MT, 2, 128, 2*IH] bf16
        out[m] = Y[mt[m], ch[m], p[m],
                   2 * slot[m] + half[m]].astype(np.float32)
    return out


if __name__ == "__main__":
    dat = np.load("/root/problem/cached_io.npz")
    inputs = {k: dat[k] for k in ("z_drug", "global_weight", "local_diag",
                                  "batch_edges", "edge_sub_type_idx")}
    expected = dat["expected"]
    actual = kernel(**inputs)
    err = np.abs(actual - expected)
    print("max abs err:", err.max(), "mean:", err.mean())
    print("Relative error:", err.max() / np.abs(expected).max())
